# revision 1
# baseline (speedup 1.0000x reference)
"""Trainium2 Bass kernel for nn_Decoder_70549132804202.

4-layer LSTM decoder (B=32, T=64, H=E=512) + 32000-way classifier with
log_softmax over the sequence axis, SPMD across 8 NeuronCores.

Phase-1 distribution: the LSTM scan is replicated on all 8 cores (it is
latency-bound and cheap to replicate); the classifier weight / vocab dim
is sharded 8 ways (log_softmax is over T, so it is fully local per core);
each core writes its [4096, 64*32] v-major slice of the output, assembled
on the host.

Layouts (per core):
  - free index n = t*32 + b   (t-major columns everywhere)
  - inpT / outsT: one [128, 8192] SBUF tile, cols = k*2048 + n  (h-dim
    tile k on partitions)
  - weights WihT/WhhT: [128, 8192] cols = k*2048 + gate_col, gate cols
    permuted to [i | f | o | g] so one Sigmoid op covers i,f,o.
  - gates computed B-major [32, 2048] in PSUM; h' transposed back to
    H-major via PE transposes each step.
"""

import os
import numpy as np

V, E, H, L, B, T = 32000, 512, 512, 4, 32, 64
NT = B * T                    # 2048
VPAD = 32768
NCORES = 8
VS = VPAD // NCORES           # 4096 rows/core (padded)
VREAL = V // NCORES           # 4000 real rows/core

TRACE = bool(int(os.environ.get("BASS_KERNEL_TRACE", "0")))

_cache = {}


def _split_excess_waits(nc, limit=1):
    """This container's walrus (2026-05-04) rejects instructions with more
    than one sync-wait command.  Hoist excess waits onto InstNoOp's
    inserted just before, on the same engine (engine streams execute in
    block order, so the waits still complete before the instruction)."""
    import concourse.mybir as mybir

    n_split = 0
    for bb in nc.main_func.blocks:
        il = bb.instructions
        i = 0
        while i < len(il):
            ins = il[i]
            si = getattr(ins, "sync_info", None)
            if si is not None and si.on_wait is not None and len(si.on_wait) > limit:
                waits = list(si.on_wait)
                keep, extra = waits[-limit:], waits[:-limit]
                pos = i
                for c0 in range(0, len(extra), limit):
                    chunk = extra[c0 : c0 + limit]
                    nop = mybir.InstNoOp(name=f"{ins.name}-ws{c0}", ins=[], outs=[])
                    nop.engine = ins.engine
                    nop.sync_info = mybir.SyncInfo(on_update=[], on_wait=chunk)
                    il.insert(pos, nop)
                    pos += 1
                    i += 1
                si.on_wait = keep
                n_split += 1
            i += 1
    return n_split


def _build():
    import concourse.bass as bass
    import concourse.mybir as mybir
    import concourse.tile as tile
    from concourse.masks import make_identity

    f32 = mybir.dt.float32
    i32 = mybir.dt.int32
    AF = mybir.ActivationFunctionType
    OP = mybir.AluOpType
    AX = mybir.AxisListType

    nc = bass.Bass("TRN2", target_bir_lowering=False, debug=False,
                   num_devices=NCORES)

    emb_ext = nc.declare_dram_parameter("emb", [V, E], f32, isOutput=False)
    dec_ext = nc.declare_dram_parameter("dec", [16, 128, 1], i32, isOutput=False)
    wih_ext = nc.declare_dram_parameter("wih", [L, 128, 8192], f32, isOutput=False)
    whh_ext = nc.declare_dram_parameter("whh", [L, 128, 8192], f32, isOutput=False)
    bias_ext = nc.declare_dram_parameter("bias", [L, 1, 2048], f32, isOutput=False)
    h0_ext = nc.declare_dram_parameter("h0t", [L, 128, 128], f32, isOutput=False)
    c0_ext = nc.declare_dram_parameter("c0b", [L, 32, 512], f32, isOutput=False)
    wcls_ext = nc.declare_dram_parameter("wcls", [32, 128, 512], f32, isOutput=False)

    out_ext = nc.declare_dram_parameter("outp", [32, 128, 2048], f32, isOutput=True)
    hT_ext = nc.declare_dram_parameter("hT", [L, 128, 128], f32, isOutput=True)
    cT_ext = nc.declare_dram_parameter("cT", [L, 32, 512], f32, isOutput=True)

    with tile.TileContext(nc) as tc:
        with (
            tc.tile_pool(name="inpT", bufs=2) as p_inpT,
            tc.tile_pool(name="wts", bufs=1) as p_w,
            tc.tile_pool(name="xg", bufs=3) as p_xg,
            tc.tile_pool(name="big", bufs=2) as p_big,
            tc.tile_pool(name="chain", bufs=2) as p_chain,
            tc.tile_pool(name="state", bufs=2) as p_state,
            tc.tile_pool(name="wc", bufs=2) as p_wc,
            tc.tile_pool(name="misc", bufs=1) as p_misc,
            tc.tile_pool(name="ps", bufs=1, space="PSUM") as p_ps,
            tc.tile_pool(name="pstr", bufs=2, space="PSUM") as p_pstr,
            tc.tile_pool(name="dram", bufs=2, space="DRAM") as p_dram,
        ):
            ident = p_misc.tile([128, 128], f32, tag="ident")
            make_identity(nc, ident[:])
            ones = p_misc.tile([1, 128], f32, tag="ones")
            nc.gpsimd.memset(ones[:], 1.0)
            bias_sb = p_misc.tile([1, 2048], f32, tag="bias")

            # ---- Stage A: embedding gather + transpose -> inpT (layer-0 input) ----
            inpT = p_inpT.tile([128, 8192], f32, tag="inpT")
            for g in range(16):
                idx = p_wc.tile([128, 1], i32, tag="idx")
                nc.sync.dma_start(idx[:], dec_ext[g])
                xr = p_big.tile([128, 512], f32, tag="big")
                nc.gpsimd.indirect_dma_start(
                    out=xr[:], out_offset=None,
                    in_=emb_ext[:],
                    in_offset=bass.IndirectOffsetOnAxis(ap=idx[:, :1], axis=0),
                )
                for k in range(4):
                    pst = p_pstr.tile([128, 128], f32, tag="pstr")
                    nc.tensor.transpose(out=pst[:], in_=xr[:, k * 128:(k + 1) * 128],
                                        identity=ident[:])
                    nc.vector.tensor_copy(
                        out=inpT[:, k * 2048 + g * 128: k * 2048 + g * 128 + 128],
                        in_=pst[:])

            # ---- layers ----
            for l in range(L):
                # B1: batched x-side gates -> DRAM (rows n = t*32+b)
                w_sb = p_w.tile([128, 8192], f32, tag="w")
                nc.sync.dma_start(w_sb[:], wih_ext[l])
                nc.sync.dma_start(bias_sb[:], bias_ext[l])
                xgd = p_dram.tile([16, 128, 2048], f32, tag="xg")
                for mc in range(16):
                    ps = p_ps.tile([128, 2048], f32, tag="ps")
                    for n in range(4):
                        ns = slice(n * 512, (n + 1) * 512)
                        for k in range(4):
                            nc.tensor.matmul(
                                ps[:, ns],
                                lhsT=inpT[:, k * 2048 + mc * 128: k * 2048 + mc * 128 + 128],
                                rhs=w_sb[:, k * 2048 + n * 512: k * 2048 + (n + 1) * 512],
                                start=(k == 0), stop=False)
                        nc.tensor.matmul(ps[:, ns], lhsT=ones[0:1, 0:128],
                                         rhs=bias_sb[0:1, ns], start=False, stop=True)
                    st = p_big.tile([128, 2048], f32, tag="big")
                    nc.scalar.copy(st[:], ps[:])
                    nc.sync.dma_start(xgd[mc], st[:])

                # B2: recurrent scan
                w_sb = p_w.tile([128, 8192], f32, tag="w")
                nc.sync.dma_start(w_sb[:], whh_ext[l])
                hT = p_state.tile([128, 128], f32, tag="hT")
                nc.sync.dma_start(hT[:], h0_ext[l])
                cB = p_state.tile([32, 512], f32, tag="cB")
                nc.sync.dma_start(cB[:], c0_ext[l])
                inpT_next = p_inpT.tile([128, 8192], f32, tag="inpT")

                for t in range(T):
                    xg = p_xg.tile([32, 2048], f32, tag="xg")
                    nc.sync.dma_start(
                        xg[:], xgd[t // 4, (t % 4) * 32:(t % 4) * 32 + 32, :])
                    ps = p_ps.tile([32, 2048], f32, tag="ps")
                    for n in range(4):
                        ns = slice(n * 512, (n + 1) * 512)
                        for k in range(4):
                            nc.tensor.matmul(
                                ps[:, ns], lhsT=hT[:, k * 32:(k + 1) * 32],
                                rhs=w_sb[:, k * 2048 + n * 512: k * 2048 + (n + 1) * 512],
                                start=(k == 0), stop=False)
                        nc.tensor.matmul(ps[:, ns], lhsT=ident[0:32, 0:32],
                                         rhs=xg[:, ns], start=False, stop=True)
                    # gate math (gate cols permuted to [i | f | o | g])
                    sig = p_chain.tile([32, 1536], f32, tag="sig")
                    nc.scalar.activation(sig[:], ps[:, 0:1536], AF.Sigmoid)
                    tg = p_chain.tile([32, 512], f32, tag="tg")
                    nc.scalar.activation(tg[:], ps[:, 1536:2048], AF.Tanh)
                    m1 = p_chain.tile([32, 512], f32, tag="m1")
                    nc.vector.tensor_tensor(out=m1[:], in0=sig[:, 512:1024],
                                            in1=cB[:], op=OP.mult)
                    m2 = p_chain.tile([32, 512], f32, tag="m2")
                    nc.vector.tensor_tensor(out=m2[:], in0=sig[:, 0:512],
                                            in1=tg[:], op=OP.mult)
                    cB = p_state.tile([32, 512], f32, tag="cB")
                    nc.vector.tensor_tensor(out=cB[:], in0=m1[:], in1=m2[:], op=OP.add)
                    tcl = p_chain.tile([32, 512], f32, tag="tc")
                    nc.scalar.activation(tcl[:], cB[:], AF.Tanh)
                    hp = p_chain.tile([32, 512], f32, tag="hp")
                    nc.vector.tensor_tensor(out=hp[:], in0=sig[:, 1024:1536],
                                            in1=tcl[:], op=OP.mult)
                    # h' -> H-major (new hT) and residual add -> inpT_next cols
                    pst = p_pstr.tile([128, 128], f32, tag="pstr")
                    for k in range(4):
                        nc.tensor.transpose(out=pst[:, k * 32:(k + 1) * 32],
                                            in_=hp[:, k * 128:(k + 1) * 128],
                                            identity=ident[0:32, 0:32])
                    hT = p_state.tile([128, 128], f32, tag="hT")
                    nc.vector.tensor_copy(out=hT[:], in_=pst[:])
                    pst_v = pst[:].rearrange("p (k n) -> p k n", k=4)
                    in_v = inpT[:].rearrange("p (k n) -> p k n", k=4)[:, :, t * 32:(t + 1) * 32]
                    out_v = inpT_next[:].rearrange("p (k n) -> p k n", k=4)[:, :, t * 32:(t + 1) * 32]
                    nc.vector.tensor_tensor(out=out_v, in0=pst_v, in1=in_v, op=OP.add)

                nc.sync.dma_start(hT_ext[l], hT[:])
                nc.sync.dma_start(cT_ext[l], cB[:])
                inpT = inpT_next

            # ---- classifier + log_softmax over T (V-sharded; b_cls cancels) ----
            for vt in range(32):
                wc = p_wc.tile([128, 512], f32, tag="wc")
                nc.sync.dma_start(wc[:], wcls_ext[vt])
                ps = p_ps.tile([128, 2048], f32, tag="ps")
                for n in range(4):
                    ns = slice(n * 512, (n + 1) * 512)
                    for k in range(4):
                        nc.tensor.matmul(
                            ps[:, ns], lhsT=wc[:, k * 128:(k + 1) * 128],
                            rhs=inpT[:, k * 2048 + n * 512: k * 2048 + (n + 1) * 512],
                            start=(k == 0), stop=(k == 3))
                ex = p_big.tile([128, 2048], f32, tag="big")
                nc.scalar.activation(ex[:], ps[:], AF.Exp)
                ls = p_wc.tile([128, 32], f32, tag="ls")
                nc.vector.tensor_reduce(out=ls[:],
                                        in_=ex[:].rearrange("p (t b) -> p b t", b=32),
                                        axis=AX.X, op=OP.add)
                lse = p_wc.tile([128, 32], f32, tag="lse")
                nc.scalar.activation(lse[:], ls[:], AF.Ln)
                nc.vector.tensor_tensor(
                    out=ex[:].rearrange("p (t b) -> p b t", b=32),
                    in0=ps[:].rearrange("p (t b) -> p b t", b=32),
                    in1=lse[:].to_broadcast([128, 32, 64]),
                    op=OP.subtract)
                nc.sync.dma_start(out_ext[vt], ex[:])

    _split_excess_waits(nc, 1)
    return nc


def _get_nc():
    if "nc" not in _cache:
        _cache["nc"] = _build()
    return _cache["nc"]


def _pack_kT(WT):
    """[512, C] (h-dim major) -> [128, 4*C] with cols = k*C + c."""
    C = WT.shape[1]
    return np.ascontiguousarray(
        WT.reshape(4, 128, C).transpose(1, 0, 2).reshape(128, 4 * C))


def kernel(context=None, dec_input=None, h0=None, c0=None, emb=None,
           W_ih=None, W_hh=None, b_ih=None, b_hh=None, W_cls=None,
           b_cls=None):
    from concourse.bass_utils import run_bass_kernel_spmd

    nc = _get_nc()

    emb = np.ascontiguousarray(np.asarray(emb, np.float32))
    dec = np.asarray(dec_input)
    h0 = np.asarray(h0, np.float32)
    c0 = np.ascontiguousarray(np.asarray(c0, np.float32))
    W_ih = np.asarray(W_ih, np.float32)
    W_hh = np.asarray(W_hh, np.float32)
    b_ih = np.asarray(b_ih, np.float32)
    b_hh = np.asarray(b_hh, np.float32)
    W_cls = np.asarray(W_cls, np.float32)

    # gate permutation: torch order [i f g o] -> kernel order [i f o g]
    perm = np.concatenate([np.arange(0, 512), np.arange(512, 1024),
                           np.arange(1536, 2048), np.arange(1024, 1536)])

    wih_pack = np.stack([_pack_kT(W_ih[l][perm].T) for l in range(L)])
    whh_pack = np.stack([_pack_kT(W_hh[l][perm].T) for l in range(L)])
    bias_pack = np.ascontiguousarray(
        (b_ih + b_hh)[:, perm].reshape(L, 1, 2048).astype(np.float32))
    h0t_pack = np.stack([_pack_kT(h0[l].T) for l in range(L)])  # [L,128,128]

    # n = t*32 + b  ->  dec.T flattened
    dec_pack = np.ascontiguousarray(
        dec.T.reshape(16, 128, 1).astype(np.int32))

    in_maps = []
    for c in range(NCORES):
        shard = np.zeros((VS, H), np.float32)       # [4096, 512]
        shard[:VREAL] = W_cls[c * VREAL:(c + 1) * VREAL]
        wcls_pack = np.stack([_pack_kT(shard[vt * 128:(vt + 1) * 128].T)
                              for vt in range(32)])  # [32, 128, 512]
        in_maps.append({
            "emb": emb, "dec": dec_pack, "wih": wih_pack, "whh": whh_pack,
            "bias": bias_pack, "h0t": h0t_pack, "c0b": c0,
            "wcls": wcls_pack,
        })

    res = run_bass_kernel_spmd(nc, in_maps, list(range(NCORES)), trace=TRACE)
    if TRACE and res.exec_time_ns is not None:
        print(f"HW exec time: {res.exec_time_ns} ns")

    out_full = np.empty((B, T, V), np.float32)
    for c in range(NCORES):
        lt = res.results[c]["outp"].reshape(VS, T, B)   # [v, t, b]
        out_full[:, :, c * VREAL:(c + 1) * VREAL] = lt[:VREAL].transpose(2, 1, 0)

    hT_buf = res.results[0]["hT"]                       # [L, 128, 128]
    h_out = np.stack([
        hT_buf[l].reshape(128, 4, 32).transpose(2, 1, 0).reshape(32, 512)
        for l in range(L)])
    c_out = res.results[0]["cT"]                        # [L, 32, 512]

    return (out_full, h_out, c_out)


# revision 3
# speedup vs baseline: 1.1111x; 1.1111x over previous
"""Trainium2 Bass kernel for nn_Decoder_70549132804202.

4-layer LSTM decoder (B=32, T=64, H=E=512) + 32000-way classifier with
log_softmax over the sequence axis, SPMD across 8 NeuronCores.

Phase-1 distribution: the LSTM scan is replicated on all 8 cores (it is
latency-bound and cheap to replicate); the classifier weight / vocab dim
is sharded 8 ways (log_softmax is over T, so it is fully local per core);
each core writes its [4096, 64*32] v-major slice of the output, assembled
on the host.

Layouts (per core):
  - free index n = t*32 + b   (t-major columns everywhere)
  - inpT / outsT: one [128, 8192] SBUF tile, cols = k*2048 + n  (h-dim
    tile k on partitions)
  - weights WihT/WhhT: [128, 8192] cols = k*2048 + gate_col, gate cols
    permuted to [i | f | o | g] so one Sigmoid op covers i,f,o.
  - gates computed B-major [32, 2048] in PSUM; h' transposed back to
    H-major via PE transposes each step.
"""

import os
import numpy as np
import ml_dtypes

BF16 = ml_dtypes.bfloat16

V, E, H, L, B, T = 32000, 512, 512, 4, 32, 64
NT = B * T                    # 2048
VPAD = 32768
NCORES = 8
VS = VPAD // NCORES           # 4096 rows/core (padded)
VREAL = V // NCORES           # 4000 real rows/core

TRACE = bool(int(os.environ.get("BASS_KERNEL_TRACE", "0")))

_cache = {}


def _split_excess_waits(nc, limit=1):
    """This container's walrus (2026-05-04) rejects instructions with more
    than one sync-wait command.  Hoist excess waits onto InstNoOp's
    inserted just before, on the same engine (engine streams execute in
    block order, so the waits still complete before the instruction)."""
    import concourse.mybir as mybir

    n_split = 0
    for bb in nc.main_func.blocks:
        il = bb.instructions
        i = 0
        while i < len(il):
            ins = il[i]
            si = getattr(ins, "sync_info", None)
            if si is not None and si.on_wait is not None and len(si.on_wait) > limit:
                waits = list(si.on_wait)
                keep, extra = waits[-limit:], waits[:-limit]
                pos = i
                for c0 in range(0, len(extra), limit):
                    chunk = extra[c0 : c0 + limit]
                    nop = mybir.InstNoOp(name=f"{ins.name}-ws{c0}", ins=[], outs=[])
                    nop.engine = ins.engine
                    nop.sync_info = mybir.SyncInfo(on_update=[], on_wait=chunk)
                    il.insert(pos, nop)
                    pos += 1
                    i += 1
                si.on_wait = keep
                n_split += 1
            i += 1
    return n_split


def _build():
    import concourse.bass as bass
    import concourse.mybir as mybir
    import concourse.tile as tile
    from concourse.masks import make_identity

    f32 = mybir.dt.float32
    bf16 = mybir.dt.bfloat16
    i32 = mybir.dt.int32
    AF = mybir.ActivationFunctionType
    OP = mybir.AluOpType
    AX = mybir.AxisListType

    nc = bass.Bass("TRN2", target_bir_lowering=False, debug=False,
                   num_devices=NCORES)

    emb_ext = nc.declare_dram_parameter("emb", [V, E], f32, isOutput=False)
    dec_ext = nc.declare_dram_parameter("dec", [16, 128, 1], i32, isOutput=False)
    wih_ext = nc.declare_dram_parameter("wih", [L, 128, 8192], bf16, isOutput=False)
    whh_ext = nc.declare_dram_parameter("whh", [L, 128, 8192], bf16, isOutput=False)
    bias_ext = nc.declare_dram_parameter("bias", [L, 1, 2048], bf16, isOutput=False)
    h0_ext = nc.declare_dram_parameter("h0t", [L, 128, 128], bf16, isOutput=False)
    c0_ext = nc.declare_dram_parameter("c0b", [L, 32, 512], f32, isOutput=False)
    wcls_ext = nc.declare_dram_parameter("wcls", [32, 128, 512], bf16, isOutput=False)

    out_ext = nc.declare_dram_parameter("outp", [32, 128, 2048], f32, isOutput=True)
    hT_ext = nc.declare_dram_parameter("hT", [L, 128, 128], f32, isOutput=True)
    cT_ext = nc.declare_dram_parameter("cT", [L, 32, 512], f32, isOutput=True)

    with tile.TileContext(nc) as tc:
        with (
            tc.tile_pool(name="inpT", bufs=2) as p_inpT,
            tc.tile_pool(name="wts", bufs=3) as p_w,
            tc.tile_pool(name="xg", bufs=4) as p_xg,
            tc.tile_pool(name="big", bufs=2) as p_big,
            tc.tile_pool(name="chain", bufs=2) as p_chain,
            tc.tile_pool(name="state", bufs=2) as p_state,
            tc.tile_pool(name="wc", bufs=2) as p_wc,
            tc.tile_pool(name="misc", bufs=1) as p_misc,
            tc.tile_pool(name="ps", bufs=1, space="PSUM") as p_ps,
            tc.tile_pool(name="pstr", bufs=2, space="PSUM") as p_pstr,
            tc.tile_pool(name="dram", bufs=2, space="DRAM") as p_dram,
        ):
            ident = p_misc.tile([128, 128], f32, tag="ident")
            make_identity(nc, ident[:])
            ones = p_misc.tile([1, 128], bf16, tag="ones")
            nc.gpsimd.memset(ones[:], 1.0)
            ident_bf = p_misc.tile([32, 32], bf16, tag="identbf")
            nc.vector.tensor_copy(ident_bf[:], ident[0:32, 0:32])
            bias_sb = p_misc.tile([1, 2048], bf16, tag="bias")

            # ---- Stage A: embedding gather + transpose -> inpT (layer-0 input) ----
            inpT = p_inpT.tile([128, 8192], bf16, tag="inpT")
            for g in range(16):
                idx = p_wc.tile([128, 1], i32, tag="idx")
                nc.sync.dma_start(idx[:], dec_ext[g])
                xr = p_big.tile([128, 512], f32, tag="big")
                nc.gpsimd.indirect_dma_start(
                    out=xr[:], out_offset=None,
                    in_=emb_ext[:],
                    in_offset=bass.IndirectOffsetOnAxis(ap=idx[:, :1], axis=0),
                )
                for k in range(4):
                    pst = p_pstr.tile([128, 128], f32, tag="pstr")
                    nc.tensor.transpose(out=pst[:], in_=xr[:, k * 128:(k + 1) * 128],
                                        identity=ident[:])
                    nc.vector.tensor_copy(
                        out=inpT[:, k * 2048 + g * 128: k * 2048 + g * 128 + 128],
                        in_=pst[:])

            # ---- layers ----
            for l in range(L):
                # B1: batched x-side gates -> DRAM (rows n = t*32+b)
                w_sb = p_w.tile([128, 8192], bf16, tag="w")
                nc.sync.dma_start(w_sb[:], wih_ext[l])
                nc.sync.dma_start(bias_sb[:], bias_ext[l])
                xgd = p_dram.tile([16, 128, 2048], bf16, tag="xg")
                for mc in range(16):
                    ps = p_ps.tile([128, 2048], f32, tag="ps")
                    for n in range(4):
                        ns = slice(n * 512, (n + 1) * 512)
                        for k in range(4):
                            nc.tensor.matmul(
                                ps[:, ns],
                                lhsT=inpT[:, k * 2048 + mc * 128: k * 2048 + mc * 128 + 128],
                                rhs=w_sb[:, k * 2048 + n * 512: k * 2048 + (n + 1) * 512],
                                start=(k == 0), stop=False)
                        nc.tensor.matmul(ps[:, ns], lhsT=ones[0:1, 0:128],
                                         rhs=bias_sb[0:1, ns], start=False, stop=True)
                    st = p_big.tile([128, 2048], bf16, tag="bigb")
                    nc.scalar.copy(st[:], ps[:])
                    nc.sync.dma_start(xgd[mc], st[:])

                # B2: recurrent scan
                w_sb = p_w.tile([128, 8192], bf16, tag="w")
                nc.sync.dma_start(w_sb[:], whh_ext[l])
                hT = p_state.tile([128, 128], bf16, tag="hT")
                nc.sync.dma_start(hT[:], h0_ext[l])
                cB = p_state.tile([32, 512], f32, tag="cB")
                nc.sync.dma_start(cB[:], c0_ext[l])
                inpT_next = p_inpT.tile([128, 8192], bf16, tag="inpT")

                for t in range(T):
                    xg = p_xg.tile([32, 2048], bf16, tag="xg")
                    nc.sync.dma_start(
                        xg[:], xgd[t // 4, (t % 4) * 32:(t % 4) * 32 + 32, :])
                    ps = p_ps.tile([32, 2048], f32, tag="ps")
                    for n in range(4):
                        ns = slice(n * 512, (n + 1) * 512)
                        for k in range(4):
                            nc.tensor.matmul(
                                ps[:, ns], lhsT=hT[:, k * 32:(k + 1) * 32],
                                rhs=w_sb[:, k * 2048 + n * 512: k * 2048 + (n + 1) * 512],
                                start=(k == 0), stop=False)
                        nc.tensor.matmul(ps[:, ns], lhsT=ident_bf[:],
                                         rhs=xg[:, ns], start=False, stop=True)
                    # gate math (gate cols permuted to [i | f | o | g])
                    sig = p_chain.tile([32, 1536], f32, tag="sig")
                    nc.scalar.activation(sig[:], ps[:, 0:1536], AF.Sigmoid)
                    tg = p_chain.tile([32, 512], f32, tag="tg")
                    nc.scalar.activation(tg[:], ps[:, 1536:2048], AF.Tanh)
                    m1 = p_chain.tile([32, 512], f32, tag="m1")
                    nc.vector.tensor_tensor(out=m1[:], in0=sig[:, 512:1024],
                                            in1=cB[:], op=OP.mult)
                    m2 = p_chain.tile([32, 512], f32, tag="m2")
                    nc.vector.tensor_tensor(out=m2[:], in0=sig[:, 0:512],
                                            in1=tg[:], op=OP.mult)
                    cB = p_state.tile([32, 512], f32, tag="cB")
                    nc.vector.tensor_tensor(out=cB[:], in0=m1[:], in1=m2[:], op=OP.add)
                    tcl = p_chain.tile([32, 512], f32, tag="tc")
                    nc.scalar.activation(tcl[:], cB[:], AF.Tanh)
                    hp = p_chain.tile([32, 512], f32, tag="hp")
                    nc.vector.tensor_tensor(out=hp[:], in0=sig[:, 1024:1536],
                                            in1=tcl[:], op=OP.mult)
                    # h' -> H-major (new hT) and residual add -> inpT_next cols
                    pst = p_pstr.tile([128, 128], f32, tag="pstr")
                    for k in range(4):
                        nc.tensor.transpose(out=pst[:, k * 32:(k + 1) * 32],
                                            in_=hp[:, k * 128:(k + 1) * 128],
                                            identity=ident[0:32, 0:32])
                    hT = p_state.tile([128, 128], bf16, tag="hT")
                    nc.vector.tensor_copy(out=hT[:], in_=pst[:])
                    pst_v = pst[:].rearrange("p (k n) -> p k n", k=4)
                    in_v = inpT[:].rearrange("p (k n) -> p k n", k=4)[:, :, t * 32:(t + 1) * 32]
                    out_v = inpT_next[:].rearrange("p (k n) -> p k n", k=4)[:, :, t * 32:(t + 1) * 32]
                    with nc.allow_low_precision("bf16 activation stream"):
                        nc.vector.tensor_tensor(out=out_v, in0=pst_v, in1=in_v, op=OP.add)
                    if t == T - 1:
                        hT_f = p_state.tile([128, 128], f32, tag="hTf")
                        nc.vector.tensor_copy(out=hT_f[:], in_=pst[:])

                nc.sync.dma_start(hT_ext[l], hT_f[:])
                nc.sync.dma_start(cT_ext[l], cB[:])
                inpT = inpT_next

            # ---- classifier + log_softmax over T (V-sharded; b_cls cancels) ----
            for vt in range(32):
                wc = p_wc.tile([128, 512], bf16, tag="wc")
                nc.sync.dma_start(wc[:], wcls_ext[vt])
                ps = p_ps.tile([128, 2048], f32, tag="ps")
                for n in range(4):
                    ns = slice(n * 512, (n + 1) * 512)
                    for k in range(4):
                        nc.tensor.matmul(
                            ps[:, ns], lhsT=wc[:, k * 128:(k + 1) * 128],
                            rhs=inpT[:, k * 2048 + n * 512: k * 2048 + (n + 1) * 512],
                            start=(k == 0), stop=(k == 3))
                ex = p_big.tile([128, 2048], f32, tag="big")
                nc.scalar.activation(ex[:], ps[:], AF.Exp)
                ls = p_wc.tile([128, 32], f32, tag="ls")
                nc.vector.tensor_reduce(out=ls[:],
                                        in_=ex[:].rearrange("p (t b) -> p b t", b=32),
                                        axis=AX.X, op=OP.add)
                lse = p_wc.tile([128, 32], f32, tag="lse")
                nc.scalar.activation(lse[:], ls[:], AF.Ln)
                nc.vector.tensor_tensor(
                    out=ex[:].rearrange("p (t b) -> p b t", b=32),
                    in0=ps[:].rearrange("p (t b) -> p b t", b=32),
                    in1=lse[:].to_broadcast([128, 32, 64]),
                    op=OP.subtract)
                nc.sync.dma_start(out_ext[vt], ex[:])

    _split_excess_waits(nc, 1)
    return nc


def _get_nc():
    if "nc" not in _cache:
        _cache["nc"] = _build()
    return _cache["nc"]


def _pack_kT(WT):
    """[512, C] (h-dim major) -> [128, 4*C] with cols = k*C + c."""
    C = WT.shape[1]
    return np.ascontiguousarray(
        WT.reshape(4, 128, C).transpose(1, 0, 2).reshape(128, 4 * C))


def kernel(context=None, dec_input=None, h0=None, c0=None, emb=None,
           W_ih=None, W_hh=None, b_ih=None, b_hh=None, W_cls=None,
           b_cls=None):
    from concourse.bass_utils import run_bass_kernel_spmd

    nc = _get_nc()

    emb = np.ascontiguousarray(np.asarray(emb, np.float32))
    dec = np.asarray(dec_input)
    h0 = np.asarray(h0, np.float32)
    c0 = np.ascontiguousarray(np.asarray(c0, np.float32))
    W_ih = np.asarray(W_ih, np.float32)
    W_hh = np.asarray(W_hh, np.float32)
    b_ih = np.asarray(b_ih, np.float32)
    b_hh = np.asarray(b_hh, np.float32)
    W_cls = np.asarray(W_cls, np.float32)

    # gate permutation: torch order [i f g o] -> kernel order [i f o g]
    perm = np.concatenate([np.arange(0, 512), np.arange(512, 1024),
                           np.arange(1536, 2048), np.arange(1024, 1536)])

    wih_pack = np.stack([_pack_kT(W_ih[l][perm].T) for l in range(L)]).astype(BF16)
    whh_pack = np.stack([_pack_kT(W_hh[l][perm].T) for l in range(L)]).astype(BF16)
    bias_pack = np.ascontiguousarray(
        (b_ih + b_hh)[:, perm].reshape(L, 1, 2048).astype(BF16))
    h0t_pack = np.stack([_pack_kT(h0[l].T) for l in range(L)]).astype(BF16)

    # n = t*32 + b  ->  dec.T flattened
    dec_pack = np.ascontiguousarray(
        dec.T.reshape(16, 128, 1).astype(np.int32))

    in_maps = []
    for c in range(NCORES):
        shard = np.zeros((VS, H), np.float32)       # [4096, 512]
        shard[:VREAL] = W_cls[c * VREAL:(c + 1) * VREAL]
        wcls_pack = np.stack([_pack_kT(shard[vt * 128:(vt + 1) * 128].T)
                              for vt in range(32)]).astype(BF16)
        in_maps.append({
            "emb": emb, "dec": dec_pack, "wih": wih_pack, "whh": whh_pack,
            "bias": bias_pack, "h0t": h0t_pack, "c0b": c0,
            "wcls": wcls_pack,
        })

    res = run_bass_kernel_spmd(nc, in_maps, list(range(NCORES)), trace=TRACE)
    if TRACE and res.exec_time_ns is not None:
        print(f"HW exec time: {res.exec_time_ns} ns")

    out_full = np.empty((B, T, V), np.float32)
    for c in range(NCORES):
        lt = res.results[c]["outp"].reshape(VS, T, B)   # [v, t, b]
        out_full[:, :, c * VREAL:(c + 1) * VREAL] = lt[:VREAL].transpose(2, 1, 0)

    hT_buf = res.results[0]["hT"]                       # [L, 128, 128]
    h_out = np.stack([
        hT_buf[l].reshape(128, 4, 32).transpose(2, 1, 0).reshape(32, 512)
        for l in range(L)])
    c_out = res.results[0]["cT"]                        # [L, 32, 512]

    return (out_full, h_out, c_out)


# revision 4
# speedup vs baseline: 1.1237x; 1.0114x over previous
"""Trainium2 Bass kernel for nn_Decoder_70549132804202.

4-layer LSTM decoder (B=32, T=64, H=E=512) + 32000-way classifier with
log_softmax over the sequence axis, SPMD across 8 NeuronCores.

Phase-1 distribution: the LSTM scan is replicated on all 8 cores (it is
latency-bound and cheap to replicate); the classifier weight / vocab dim
is sharded 8 ways (log_softmax is over T, so it is fully local per core);
each core writes its [4096, 64*32] v-major slice of the output, assembled
on the host.

Layouts (per core):
  - free index n = t*32 + b   (t-major columns everywhere)
  - inpT / outsT: one [128, 8192] SBUF tile, cols = k*2048 + n  (h-dim
    tile k on partitions)
  - weights WihT/WhhT: [128, 8192] cols = k*2048 + gate_col, gate cols
    permuted to [i | f | o | g] so one Sigmoid op covers i,f,o.
  - gates computed B-major [32, 2048] in PSUM; h' transposed back to
    H-major via PE transposes each step.
"""

import os
import numpy as np
import ml_dtypes

BF16 = ml_dtypes.bfloat16

V, E, H, L, B, T = 32000, 512, 512, 4, 32, 64
NT = B * T                    # 2048
VPAD = 32768
NCORES = 8
VS = VPAD // NCORES           # 4096 rows/core (padded)
VREAL = V // NCORES           # 4000 real rows/core

TRACE = bool(int(os.environ.get("BASS_KERNEL_TRACE", "0")))

_cache = {}


def _split_excess_waits(nc, limit=1):
    """This container's walrus (2026-05-04) rejects instructions with more
    than one sync-wait command.  Hoist excess waits onto InstNoOp's
    inserted just before, on the same engine (engine streams execute in
    block order, so the waits still complete before the instruction)."""
    import concourse.mybir as mybir

    n_split = 0
    for bb in nc.main_func.blocks:
        il = bb.instructions
        i = 0
        while i < len(il):
            ins = il[i]
            si = getattr(ins, "sync_info", None)
            if si is not None and si.on_wait is not None and len(si.on_wait) > limit:
                waits = list(si.on_wait)
                keep, extra = waits[-limit:], waits[:-limit]
                pos = i
                for c0 in range(0, len(extra), limit):
                    chunk = extra[c0 : c0 + limit]
                    nop = mybir.InstNoOp(name=f"{ins.name}-ws{c0}", ins=[], outs=[])
                    nop.engine = ins.engine
                    nop.sync_info = mybir.SyncInfo(on_update=[], on_wait=chunk)
                    il.insert(pos, nop)
                    pos += 1
                    i += 1
                si.on_wait = keep
                n_split += 1
            i += 1
    return n_split


def _build():
    import concourse.bass as bass
    import concourse.mybir as mybir
    import concourse.tile as tile
    from concourse.masks import make_identity

    f32 = mybir.dt.float32
    bf16 = mybir.dt.bfloat16
    i32 = mybir.dt.int32
    AF = mybir.ActivationFunctionType
    OP = mybir.AluOpType
    AX = mybir.AxisListType

    nc = bass.Bass("TRN2", target_bir_lowering=False, debug=False,
                   num_devices=NCORES)

    emb_ext = nc.declare_dram_parameter("emb", [V, E], f32, isOutput=False)
    dec_ext = nc.declare_dram_parameter("dec", [16, 128, 1], i32, isOutput=False)
    wih_ext = nc.declare_dram_parameter("wih", [L, 128, 8192], bf16, isOutput=False)
    whh_ext = nc.declare_dram_parameter("whh", [L, 128, 8192], bf16, isOutput=False)
    bias_ext = nc.declare_dram_parameter("bias", [L, 1, 2048], bf16, isOutput=False)
    h0_ext = nc.declare_dram_parameter("h0t", [L, 128, 128], bf16, isOutput=False)
    c0_ext = nc.declare_dram_parameter("c0b", [L, 32, 512], f32, isOutput=False)
    wcls_ext = nc.declare_dram_parameter("wcls", [32, 128, 512], bf16, isOutput=False)

    out_ext = nc.declare_dram_parameter("outp", [32, 128, 2048], f32, isOutput=True)
    hT_ext = nc.declare_dram_parameter("hT", [L, 128, 128], f32, isOutput=True)
    cT_ext = nc.declare_dram_parameter("cT", [L, 32, 512], f32, isOutput=True)

    with tile.TileContext(nc) as tc:
        with (
            tc.tile_pool(name="inpT", bufs=2) as p_inpT,
            tc.tile_pool(name="wts", bufs=2) as p_w,
            tc.tile_pool(name="xg", bufs=4) as p_xg,
            tc.tile_pool(name="big", bufs=3) as p_big,
            tc.tile_pool(name="chain", bufs=2) as p_chain,
            tc.tile_pool(name="state", bufs=2) as p_state,
            tc.tile_pool(name="wc", bufs=2) as p_wc,
            tc.tile_pool(name="misc", bufs=1) as p_misc,
            tc.tile_pool(name="ps", bufs=1, space="PSUM") as p_ps,
            tc.tile_pool(name="pstr", bufs=2, space="PSUM") as p_pstr,
            tc.tile_pool(name="psb", bufs=1, space="PSUM") as p_psb,
            tc.tile_pool(name="dram", bufs=2, space="DRAM") as p_dram,
        ):
            ident = p_misc.tile([128, 128], f32, tag="ident")
            make_identity(nc, ident[:])
            ones = p_misc.tile([1, 128], bf16, tag="ones")
            nc.gpsimd.memset(ones[:], 1.0)
            ident_bf = p_misc.tile([32, 32], bf16, tag="identbf")
            nc.vector.tensor_copy(ident_bf[:], ident[0:32, 0:32])
            bias_sb = p_misc.tile([1, 2048], bf16, tag="bias")

            # ---- Stage A: embedding gather + transpose -> inpT (layer-0 input) ----
            inpT = p_inpT.tile([128, 8192], bf16, tag="inpT")
            for g in range(16):
                idx = p_wc.tile([128, 1], i32, tag="idx")
                nc.sync.dma_start(idx[:], dec_ext[g])
                xr = p_big.tile([128, 512], f32, tag="big")
                nc.gpsimd.indirect_dma_start(
                    out=xr[:], out_offset=None,
                    in_=emb_ext[:],
                    in_offset=bass.IndirectOffsetOnAxis(ap=idx[:, :1], axis=0),
                )
                for k in range(4):
                    pst = p_pstr.tile([128, 128], f32, tag="pstr")
                    nc.tensor.transpose(out=pst[:], in_=xr[:, k * 128:(k + 1) * 128],
                                        identity=ident[:])
                    nc.vector.tensor_copy(
                        out=inpT[:, k * 2048 + g * 128: k * 2048 + g * 128 + 128],
                        in_=pst[:])

            # ---- layers ----
            # B1 (batched x-side gates) for layer l+1 is interleaved into
            # layer l's scan so PE fills the recurrence gaps.
            def emit_b1_chunk(mc, src_inpT, w_sb, xgd):
                for half in range(2):
                    psb = p_psb.tile([128, 1024], f32, tag="psb")
                    for n2 in range(2):
                        n = half * 2 + n2
                        nsl = slice(n2 * 512, (n2 + 1) * 512)
                        for k in range(4):
                            nc.tensor.matmul(
                                psb[:, nsl],
                                lhsT=src_inpT[:, k * 2048 + mc * 128: k * 2048 + mc * 128 + 128],
                                rhs=w_sb[:, k * 2048 + n * 512: k * 2048 + (n + 1) * 512],
                                start=(k == 0), stop=False)
                        nc.tensor.matmul(psb[:, nsl], lhsT=ones[0:1, 0:128],
                                         rhs=bias_sb[0:1, n * 512:(n + 1) * 512],
                                         start=False, stop=True)
                    st = p_big.tile([128, 1024], bf16, tag="bigb")
                    nc.scalar.copy(st[:], psb[:])
                    nc.sync.dma_start(xgd[mc, :, half * 1024:(half + 1) * 1024], st[:])

            # layer-0 x-gates up front
            w_ih_sb = p_w.tile([128, 8192], bf16, tag="w")
            nc.sync.dma_start(w_ih_sb[:], wih_ext[0])
            nc.sync.dma_start(bias_sb[:], bias_ext[0])
            xgd = p_dram.tile([16, 128, 2048], bf16, tag="xg")
            for mc in range(16):
                emit_b1_chunk(mc, inpT, w_ih_sb, xgd)

            for l in range(L):
                # B2: recurrent scan for layer l (+ interleaved B1 for l+1)
                w_sb = p_w.tile([128, 8192], bf16, tag="w")
                nc.sync.dma_start(w_sb[:], whh_ext[l])
                hT = p_state.tile([128, 128], bf16, tag="hT")
                nc.sync.dma_start(hT[:], h0_ext[l])
                cB = p_state.tile([32, 512], f32, tag="cB")
                nc.sync.dma_start(cB[:], c0_ext[l])
                inpT_next = p_inpT.tile([128, 8192], bf16, tag="inpT")
                if l + 1 < L:
                    w_ih_sb = p_w.tile([128, 8192], bf16, tag="w")
                    nc.sync.dma_start(w_ih_sb[:], wih_ext[l + 1])
                    nc.sync.dma_start(bias_sb[:], bias_ext[l + 1])
                    xgd_next = p_dram.tile([16, 128, 2048], bf16, tag="xg")

                for t in range(T):
                    xg = p_xg.tile([32, 2048], bf16, tag="xg")
                    nc.sync.dma_start(
                        xg[:], xgd[t // 4, (t % 4) * 32:(t % 4) * 32 + 32, :])
                    ps = p_ps.tile([32, 2048], f32, tag="ps")
                    for n in range(4):
                        ns = slice(n * 512, (n + 1) * 512)
                        for k in range(4):
                            nc.tensor.matmul(
                                ps[:, ns], lhsT=hT[:, k * 32:(k + 1) * 32],
                                rhs=w_sb[:, k * 2048 + n * 512: k * 2048 + (n + 1) * 512],
                                start=(k == 0), stop=False)
                        nc.tensor.matmul(ps[:, ns], lhsT=ident_bf[:],
                                         rhs=xg[:, ns], start=False, stop=True)
                    # gate math (gate cols permuted to [i | f | o | g])
                    sig = p_chain.tile([32, 1536], f32, tag="sig")
                    nc.scalar.activation(sig[:], ps[:, 0:1536], AF.Sigmoid)
                    tg = p_chain.tile([32, 512], f32, tag="tg")
                    nc.scalar.activation(tg[:], ps[:, 1536:2048], AF.Tanh)
                    m1 = p_chain.tile([32, 512], f32, tag="m1")
                    nc.vector.tensor_tensor(out=m1[:], in0=sig[:, 512:1024],
                                            in1=cB[:], op=OP.mult)
                    m2 = p_chain.tile([32, 512], f32, tag="m2")
                    nc.vector.tensor_tensor(out=m2[:], in0=sig[:, 0:512],
                                            in1=tg[:], op=OP.mult)
                    cB = p_state.tile([32, 512], f32, tag="cB")
                    nc.vector.tensor_tensor(out=cB[:], in0=m1[:], in1=m2[:], op=OP.add)
                    tcl = p_chain.tile([32, 512], f32, tag="tc")
                    nc.scalar.activation(tcl[:], cB[:], AF.Tanh)
                    hp = p_chain.tile([32, 512], f32, tag="hp")
                    nc.vector.tensor_tensor(out=hp[:], in0=sig[:, 1024:1536],
                                            in1=tcl[:], op=OP.mult)
                    # h' -> H-major (new hT) and residual add -> inpT_next cols
                    pst = p_pstr.tile([128, 128], f32, tag="pstr")
                    for k in range(4):
                        nc.tensor.transpose(out=pst[:, k * 32:(k + 1) * 32],
                                            in_=hp[:, k * 128:(k + 1) * 128],
                                            identity=ident[0:32, 0:32])
                    hT = p_state.tile([128, 128], bf16, tag="hT")
                    nc.vector.tensor_copy(out=hT[:], in_=pst[:])
                    pst_v = pst[:].rearrange("p (k n) -> p k n", k=4)
                    in_v = inpT[:].rearrange("p (k n) -> p k n", k=4)[:, :, t * 32:(t + 1) * 32]
                    out_v = inpT_next[:].rearrange("p (k n) -> p k n", k=4)[:, :, t * 32:(t + 1) * 32]
                    with nc.allow_low_precision("bf16 activation stream"):
                        nc.vector.tensor_tensor(out=out_v, in0=pst_v, in1=in_v, op=OP.add)
                    if t == T - 1:
                        hT_f = p_state.tile([128, 128], f32, tag="hTf")
                        nc.vector.tensor_copy(out=hT_f[:], in_=pst[:])
                    if l + 1 < L and t % 4 == 3:
                        emit_b1_chunk(t // 4, inpT_next, w_ih_sb, xgd_next)

                nc.sync.dma_start(hT_ext[l], hT_f[:])
                nc.sync.dma_start(cT_ext[l], cB[:])
                inpT = inpT_next
                if l + 1 < L:
                    xgd = xgd_next

            # ---- classifier + log_softmax over T (V-sharded; b_cls cancels) ----
            for vt in range(32):
                wc = p_wc.tile([128, 512], bf16, tag="wc")
                nc.sync.dma_start(wc[:], wcls_ext[vt])
                ps = p_ps.tile([128, 2048], f32, tag="ps")
                for n in range(4):
                    ns = slice(n * 512, (n + 1) * 512)
                    for k in range(4):
                        nc.tensor.matmul(
                            ps[:, ns], lhsT=wc[:, k * 128:(k + 1) * 128],
                            rhs=inpT[:, k * 2048 + n * 512: k * 2048 + (n + 1) * 512],
                            start=(k == 0), stop=(k == 3))
                lg = p_big.tile([128, 2048], f32, tag="big")
                nc.scalar.copy(lg[:], ps[:])
                ex = p_big.tile([128, 2048], f32, tag="big")
                nc.scalar.activation(ex[:], lg[:], AF.Exp)
                ls = p_wc.tile([128, 32], f32, tag="ls")
                nc.vector.tensor_reduce(out=ls[:],
                                        in_=ex[:].rearrange("p (t b) -> p b t", b=32),
                                        axis=AX.X, op=OP.add)
                lse = p_wc.tile([128, 32], f32, tag="lse")
                nc.scalar.activation(lse[:], ls[:], AF.Ln)
                nc.vector.tensor_tensor(
                    out=ex[:].rearrange("p (t b) -> p b t", b=32),
                    in0=lg[:].rearrange("p (t b) -> p b t", b=32),
                    in1=lse[:].to_broadcast([128, 32, 64]),
                    op=OP.subtract)
                nc.sync.dma_start(out_ext[vt], ex[:])

    _split_excess_waits(nc, 1)
    return nc


def _get_nc():
    if "nc" not in _cache:
        _cache["nc"] = _build()
    return _cache["nc"]


def _pack_kT(WT):
    """[512, C] (h-dim major) -> [128, 4*C] with cols = k*C + c."""
    C = WT.shape[1]
    return np.ascontiguousarray(
        WT.reshape(4, 128, C).transpose(1, 0, 2).reshape(128, 4 * C))


def kernel(context=None, dec_input=None, h0=None, c0=None, emb=None,
           W_ih=None, W_hh=None, b_ih=None, b_hh=None, W_cls=None,
           b_cls=None):
    from concourse.bass_utils import run_bass_kernel_spmd

    nc = _get_nc()

    emb = np.ascontiguousarray(np.asarray(emb, np.float32))
    dec = np.asarray(dec_input)
    h0 = np.asarray(h0, np.float32)
    c0 = np.ascontiguousarray(np.asarray(c0, np.float32))
    W_ih = np.asarray(W_ih, np.float32)
    W_hh = np.asarray(W_hh, np.float32)
    b_ih = np.asarray(b_ih, np.float32)
    b_hh = np.asarray(b_hh, np.float32)
    W_cls = np.asarray(W_cls, np.float32)

    # gate permutation: torch order [i f g o] -> kernel order [i f o g]
    perm = np.concatenate([np.arange(0, 512), np.arange(512, 1024),
                           np.arange(1536, 2048), np.arange(1024, 1536)])

    wih_pack = np.stack([_pack_kT(W_ih[l][perm].T) for l in range(L)]).astype(BF16)
    whh_pack = np.stack([_pack_kT(W_hh[l][perm].T) for l in range(L)]).astype(BF16)
    bias_pack = np.ascontiguousarray(
        (b_ih + b_hh)[:, perm].reshape(L, 1, 2048).astype(BF16))
    h0t_pack = np.stack([_pack_kT(h0[l].T) for l in range(L)]).astype(BF16)

    # n = t*32 + b  ->  dec.T flattened
    dec_pack = np.ascontiguousarray(
        dec.T.reshape(16, 128, 1).astype(np.int32))

    in_maps = []
    for c in range(NCORES):
        shard = np.zeros((VS, H), np.float32)       # [4096, 512]
        shard[:VREAL] = W_cls[c * VREAL:(c + 1) * VREAL]
        wcls_pack = np.stack([_pack_kT(shard[vt * 128:(vt + 1) * 128].T)
                              for vt in range(32)]).astype(BF16)
        in_maps.append({
            "emb": emb, "dec": dec_pack, "wih": wih_pack, "whh": whh_pack,
            "bias": bias_pack, "h0t": h0t_pack, "c0b": c0,
            "wcls": wcls_pack,
        })

    res = run_bass_kernel_spmd(nc, in_maps, list(range(NCORES)), trace=TRACE)
    if TRACE and res.exec_time_ns is not None:
        print(f"HW exec time: {res.exec_time_ns} ns")

    out_full = np.empty((B, T, V), np.float32)
    for c in range(NCORES):
        lt = res.results[c]["outp"].reshape(VS, T, B)   # [v, t, b]
        out_full[:, :, c * VREAL:(c + 1) * VREAL] = lt[:VREAL].transpose(2, 1, 0)

    hT_buf = res.results[0]["hT"]                       # [L, 128, 128]
    h_out = np.stack([
        hT_buf[l].reshape(128, 4, 32).transpose(2, 1, 0).reshape(32, 512)
        for l in range(L)])
    c_out = res.results[0]["cT"]                        # [L, 32, 512]

    return (out_full, h_out, c_out)


# revision 6
# speedup vs baseline: 1.3176x; 1.1726x over previous
"""Trainium2 Bass kernel for nn_Decoder_70549132804202.

4-layer LSTM decoder (B=32, T=64, H=E=512) + 32000-way classifier with
log_softmax over the sequence axis, SPMD across 8 NeuronCores.

Phase-1 distribution: the LSTM scan is replicated on all 8 cores (it is
latency-bound and cheap to replicate); the classifier weight / vocab dim
is sharded 8 ways (log_softmax is over T, so it is fully local per core);
each core writes its [4096, 64*32] v-major slice of the output, assembled
on the host.

Layouts (per core):
  - free index n = t*32 + b   (t-major columns everywhere)
  - inpT / outsT: one [128, 8192] SBUF tile, cols = k*2048 + n  (h-dim
    tile k on partitions)
  - weights WihT/WhhT: [128, 8192] cols = k*2048 + gate_col, gate cols
    permuted to [i | f | o | g] so one Sigmoid op covers i,f,o.
  - gates computed B-major [32, 2048] in PSUM; h' transposed back to
    H-major via PE transposes each step.
"""

import os
import numpy as np
import ml_dtypes

BF16 = ml_dtypes.bfloat16

V, E, H, L, B, T = 32000, 512, 512, 4, 32, 64
NT = B * T                    # 2048
VPAD = 32768
NCORES = 8
VS = VPAD // NCORES           # 4096 rows/core (padded)
VREAL = V // NCORES           # 4000 real rows/core

TRACE = bool(int(os.environ.get("BASS_KERNEL_TRACE", "0")))

_cache = {}


def _split_excess_waits(nc, limit=1):
    """This container's walrus (2026-05-04) rejects instructions with more
    than one sync-wait command.  Hoist excess waits onto InstNoOp's
    inserted just before, on the same engine (engine streams execute in
    block order, so the waits still complete before the instruction)."""
    import concourse.mybir as mybir

    n_split = 0
    for bb in nc.main_func.blocks:
        il = bb.instructions
        i = 0
        while i < len(il):
            ins = il[i]
            si = getattr(ins, "sync_info", None)
            if si is not None and si.on_wait is not None and len(si.on_wait) > limit:
                waits = list(si.on_wait)
                keep, extra = waits[-limit:], waits[:-limit]
                pos = i
                for c0 in range(0, len(extra), limit):
                    chunk = extra[c0 : c0 + limit]
                    nop = mybir.InstNoOp(name=f"{ins.name}-ws{c0}", ins=[], outs=[])
                    nop.engine = ins.engine
                    nop.sync_info = mybir.SyncInfo(on_update=[], on_wait=chunk)
                    il.insert(pos, nop)
                    pos += 1
                    i += 1
                si.on_wait = keep
                n_split += 1
            i += 1
    return n_split


def _build():
    import concourse.bass as bass
    import concourse.mybir as mybir
    import concourse.tile as tile
    from concourse.masks import make_identity

    f32 = mybir.dt.float32
    bf16 = mybir.dt.bfloat16
    i32 = mybir.dt.int32
    AF = mybir.ActivationFunctionType
    OP = mybir.AluOpType
    AX = mybir.AxisListType

    nc = bass.Bass("TRN2", target_bir_lowering=False, debug=False,
                   num_devices=NCORES)

    emb_ext = nc.declare_dram_parameter("emb", [V, E], f32, isOutput=False)
    dec_ext = nc.declare_dram_parameter("dec", [16, 128, 1], i32, isOutput=False)
    wih_ext = nc.declare_dram_parameter("wih", [L, 128, 8192], bf16, isOutput=False)
    whh_ext = nc.declare_dram_parameter("whh", [L, 128, 8192], bf16, isOutput=False)
    bias_ext = nc.declare_dram_parameter("bias", [L, 1, 2048], bf16, isOutput=False)
    h0_ext = nc.declare_dram_parameter("h0t", [L, 128, 128], bf16, isOutput=False)
    c0_ext = nc.declare_dram_parameter("c0b", [L, 32, 512], f32, isOutput=False)
    wcls_ext = nc.declare_dram_parameter("wcls", [32, 128, 512], bf16, isOutput=False)

    out_ext = nc.declare_dram_parameter("outp", [32, 128, 2048], f32, isOutput=True)
    hT_ext = nc.declare_dram_parameter("hT", [L, 128, 128], f32, isOutput=True)
    cT_ext = nc.declare_dram_parameter("cT", [L, 32, 512], f32, isOutput=True)

    with tile.TileContext(nc) as tc:
        with (
            tc.tile_pool(name="inpT", bufs=2) as p_inpT,
            tc.tile_pool(name="wts", bufs=2) as p_w,
            tc.tile_pool(name="xg", bufs=4) as p_xg,
            tc.tile_pool(name="big", bufs=3) as p_big,
            tc.tile_pool(name="chain", bufs=2) as p_chain,
            tc.tile_pool(name="state", bufs=2) as p_state,
            tc.tile_pool(name="wc", bufs=2) as p_wc,
            tc.tile_pool(name="misc", bufs=1) as p_misc,
            tc.tile_pool(name="ps", bufs=1, space="PSUM") as p_ps,
            tc.tile_pool(name="pstr", bufs=2, space="PSUM") as p_pstr,
            tc.tile_pool(name="psb", bufs=1, space="PSUM") as p_psb,
            tc.tile_pool(name="dram", bufs=2, space="DRAM") as p_dram,
        ):
            ident = p_misc.tile([128, 128], f32, tag="ident")
            make_identity(nc, ident[:])
            ones = p_misc.tile([1, 128], bf16, tag="ones")
            nc.gpsimd.memset(ones[:], 1.0)
            ident_bf = p_misc.tile([32, 32], bf16, tag="identbf")
            nc.vector.tensor_copy(ident_bf[:], ident[0:32, 0:32])
            bias_sb = p_misc.tile([1, 2048], bf16, tag="bias")

            # ---- Stage A: embedding gather + transpose -> inpT (layer-0 input) ----
            inpT = p_inpT.tile([128, 8192], bf16, tag="inpT")
            for g in range(16):
                idx = p_wc.tile([128, 1], i32, tag="idx")
                nc.sync.dma_start(idx[:], dec_ext[g])
                xr = p_big.tile([128, 512], f32, tag="big")
                nc.gpsimd.indirect_dma_start(
                    out=xr[:], out_offset=None,
                    in_=emb_ext[:],
                    in_offset=bass.IndirectOffsetOnAxis(ap=idx[:, :1], axis=0),
                )
                for k in range(4):
                    pst = p_pstr.tile([128, 128], f32, tag="pstr")
                    nc.tensor.transpose(out=pst[:], in_=xr[:, k * 128:(k + 1) * 128],
                                        identity=ident[:])
                    nc.vector.tensor_copy(
                        out=inpT[:, k * 2048 + g * 128: k * 2048 + g * 128 + 128],
                        in_=pst[:])

            # ---- layers ----
            # B1 (batched x-side gates) for layer l+1 is interleaved into
            # layer l's scan so PE fills the recurrence gaps.
            def emit_b1_chunk(mc, src_inpT, w_sb, xgd):
                for half in range(2):
                    psb = p_psb.tile([128, 1024], f32, tag="psb")
                    for n2 in range(2):
                        n = half * 2 + n2
                        nsl = slice(n2 * 512, (n2 + 1) * 512)
                        for k in range(4):
                            nc.tensor.matmul(
                                psb[:, nsl],
                                lhsT=src_inpT[:, k * 2048 + mc * 128: k * 2048 + mc * 128 + 128],
                                rhs=w_sb[:, k * 2048 + n * 512: k * 2048 + (n + 1) * 512],
                                start=(k == 0), stop=False)
                        nc.tensor.matmul(psb[:, nsl], lhsT=ones[0:1, 0:128],
                                         rhs=bias_sb[0:1, n * 512:(n + 1) * 512],
                                         start=False, stop=True)
                    st = p_big.tile([128, 1024], bf16, tag="bigb")
                    nc.scalar.copy(st[:], psb[:])
                    nc.sync.dma_start(xgd[mc, :, half * 1024:(half + 1) * 1024], st[:])

            # layer-0 x-gates up front
            w_ih_sb = p_w.tile([128, 8192], bf16, tag="w")
            nc.sync.dma_start(w_ih_sb[:], wih_ext[0])
            nc.sync.dma_start(bias_sb[:], bias_ext[0])
            xgd = p_dram.tile([16, 128, 2048], bf16, tag="xg")
            for mc in range(16):
                emit_b1_chunk(mc, inpT, w_ih_sb, xgd)

            for l in range(L):
                # B2: recurrent scan for layer l (+ interleaved B1 for l+1)
                w_sb = p_w.tile([128, 8192], bf16, tag="w")
                nc.sync.dma_start(w_sb[:], whh_ext[l])
                hT = p_state.tile([128, 128], bf16, tag="hT")
                nc.sync.dma_start(hT[:], h0_ext[l])
                cB = p_state.tile([32, 512], f32, tag="cB")
                nc.sync.dma_start(cB[:], c0_ext[l])
                inpT_next = p_inpT.tile([128, 8192], bf16, tag="inpT")
                if l + 1 < L:
                    w_ih_sb = p_w.tile([128, 8192], bf16, tag="w")
                    nc.sync.dma_start(w_ih_sb[:], wih_ext[l + 1])
                    nc.sync.dma_start(bias_sb[:], bias_ext[l + 1])
                    xgd_next = p_dram.tile([16, 128, 2048], bf16, tag="xg")

                for t in range(T):
                    xg = p_xg.tile([32, 2048], bf16, tag="xg")
                    nc.sync.dma_start(
                        xg[:], xgd[t // 4, (t % 4) * 32:(t % 4) * 32 + 32, :])
                    ps = p_ps.tile([32, 2048], f32, tag="ps")
                    for n in range(4):
                        ns = slice(n * 512, (n + 1) * 512)
                        nc.tensor.matmul(ps[:, ns], lhsT=ident_bf[:],
                                         rhs=xg[:, ns], start=True, stop=False)
                    for n in range(4):
                        ns = slice(n * 512, (n + 1) * 512)
                        for k in range(4):
                            nc.tensor.matmul(
                                ps[:, ns], lhsT=hT[:, k * 32:(k + 1) * 32],
                                rhs=w_sb[:, k * 2048 + n * 512: k * 2048 + (n + 1) * 512],
                                start=False, stop=(k == 3))
                    # gate math (gate cols permuted to [i | f | o | g])
                    sif = p_chain.tile([32, 1024], f32, tag="sif")
                    nc.scalar.activation(sif[:], ps[:, 0:1024], AF.Sigmoid)
                    tg = p_chain.tile([32, 512], f32, tag="tg")
                    nc.scalar.activation(tg[:], ps[:, 1536:2048], AF.Tanh)
                    so = p_chain.tile([32, 512], f32, tag="so")
                    nc.scalar.activation(so[:], ps[:, 1024:1536], AF.Sigmoid)
                    m1 = p_chain.tile([32, 512], f32, tag="m1")
                    nc.vector.tensor_tensor(out=m1[:], in0=sif[:, 512:1024],
                                            in1=cB[:], op=OP.mult)
                    m2 = p_chain.tile([32, 512], f32, tag="m2")
                    nc.vector.tensor_tensor(out=m2[:], in0=sif[:, 0:512],
                                            in1=tg[:], op=OP.mult)
                    cB = p_state.tile([32, 512], f32, tag="cB")
                    nc.vector.tensor_tensor(out=cB[:], in0=m1[:], in1=m2[:], op=OP.add)
                    tcl = p_chain.tile([32, 512], f32, tag="tc")
                    nc.scalar.activation(tcl[:], cB[:], AF.Tanh)
                    hp = p_chain.tile([32, 512], f32, tag="hp")
                    nc.vector.tensor_tensor(out=hp[:], in0=so[:],
                                            in1=tcl[:], op=OP.mult)
                    # h' -> H-major (new hT) and residual add -> inpT_next cols
                    pst = p_pstr.tile([128, 128], f32, tag="pstr")
                    for k in range(4):
                        nc.tensor.transpose(out=pst[:, k * 32:(k + 1) * 32],
                                            in_=hp[:, k * 128:(k + 1) * 128],
                                            identity=ident[0:32, 0:32])
                    hT = p_state.tile([128, 128], bf16, tag="hT")
                    nc.vector.tensor_copy(out=hT[:], in_=pst[:])
                    pst_v = pst[:].rearrange("p (k n) -> p k n", k=4)
                    in_v = inpT[:].rearrange("p (k n) -> p k n", k=4)[:, :, t * 32:(t + 1) * 32]
                    out_v = inpT_next[:].rearrange("p (k n) -> p k n", k=4)[:, :, t * 32:(t + 1) * 32]
                    with nc.allow_low_precision("bf16 activation stream"):
                        nc.vector.tensor_tensor(out=out_v, in0=pst_v, in1=in_v, op=OP.add)
                    if t == T - 1:
                        hT_f = p_state.tile([128, 128], f32, tag="hTf")
                        nc.vector.tensor_copy(out=hT_f[:], in_=pst[:])
                    if l + 1 < L and t % 4 == 3:
                        emit_b1_chunk(t // 4, inpT_next, w_ih_sb, xgd_next)

                nc.sync.dma_start(hT_ext[l], hT_f[:])
                nc.sync.dma_start(cT_ext[l], cB[:])
                inpT = inpT_next
                if l + 1 < L:
                    xgd = xgd_next

            # ---- classifier + log_softmax over T (V-sharded; b_cls cancels) ----
            for vt in range(32):
                wc = p_wc.tile([128, 512], bf16, tag="wc")
                nc.sync.dma_start(wc[:], wcls_ext[vt])
                ps = p_ps.tile([128, 2048], f32, tag="ps")
                for n in range(4):
                    ns = slice(n * 512, (n + 1) * 512)
                    for k in range(4):
                        nc.tensor.matmul(
                            ps[:, ns], lhsT=wc[:, k * 128:(k + 1) * 128],
                            rhs=inpT[:, k * 2048 + n * 512: k * 2048 + (n + 1) * 512],
                            start=(k == 0), stop=(k == 3))
                lg = p_big.tile([128, 2048], f32, tag="big")
                nc.scalar.copy(lg[:], ps[:])
                ex = p_big.tile([128, 2048], f32, tag="big")
                nc.scalar.activation(ex[:], lg[:], AF.Exp)
                ls = p_wc.tile([128, 32], f32, tag="ls")
                nc.vector.tensor_reduce(out=ls[:],
                                        in_=ex[:].rearrange("p (t b) -> p b t", b=32),
                                        axis=AX.X, op=OP.add)
                lse = p_wc.tile([128, 32], f32, tag="lse")
                nc.scalar.activation(lse[:], ls[:], AF.Ln)
                nc.vector.tensor_tensor(
                    out=ex[:].rearrange("p (t b) -> p b t", b=32),
                    in0=lg[:].rearrange("p (t b) -> p b t", b=32),
                    in1=lse[:].to_broadcast([128, 32, 64]),
                    op=OP.subtract)
                nc.sync.dma_start(out_ext[vt], ex[:])

    _split_excess_waits(nc, 1)
    return nc


def _get_nc():
    if "nc" not in _cache:
        _cache["nc"] = _build()
    return _cache["nc"]


def _pack_kT(WT):
    """[512, C] (h-dim major) -> [128, 4*C] with cols = k*C + c."""
    C = WT.shape[1]
    return np.ascontiguousarray(
        WT.reshape(4, 128, C).transpose(1, 0, 2).reshape(128, 4 * C))


def kernel(context=None, dec_input=None, h0=None, c0=None, emb=None,
           W_ih=None, W_hh=None, b_ih=None, b_hh=None, W_cls=None,
           b_cls=None):
    from concourse.bass_utils import run_bass_kernel_spmd

    nc = _get_nc()

    emb = np.ascontiguousarray(np.asarray(emb, np.float32))
    dec = np.asarray(dec_input)
    h0 = np.asarray(h0, np.float32)
    c0 = np.ascontiguousarray(np.asarray(c0, np.float32))
    W_ih = np.asarray(W_ih, np.float32)
    W_hh = np.asarray(W_hh, np.float32)
    b_ih = np.asarray(b_ih, np.float32)
    b_hh = np.asarray(b_hh, np.float32)
    W_cls = np.asarray(W_cls, np.float32)

    # gate permutation: torch order [i f g o] -> kernel order [i f o g]
    perm = np.concatenate([np.arange(0, 512), np.arange(512, 1024),
                           np.arange(1536, 2048), np.arange(1024, 1536)])

    wih_pack = np.stack([_pack_kT(W_ih[l][perm].T) for l in range(L)]).astype(BF16)
    whh_pack = np.stack([_pack_kT(W_hh[l][perm].T) for l in range(L)]).astype(BF16)
    bias_pack = np.ascontiguousarray(
        (b_ih + b_hh)[:, perm].reshape(L, 1, 2048).astype(BF16))
    h0t_pack = np.stack([_pack_kT(h0[l].T) for l in range(L)]).astype(BF16)

    # n = t*32 + b  ->  dec.T flattened
    dec_pack = np.ascontiguousarray(
        dec.T.reshape(16, 128, 1).astype(np.int32))

    in_maps = []
    for c in range(NCORES):
        shard = np.zeros((VS, H), np.float32)       # [4096, 512]
        shard[:VREAL] = W_cls[c * VREAL:(c + 1) * VREAL]
        wcls_pack = np.stack([_pack_kT(shard[vt * 128:(vt + 1) * 128].T)
                              for vt in range(32)]).astype(BF16)
        in_maps.append({
            "emb": emb, "dec": dec_pack, "wih": wih_pack, "whh": whh_pack,
            "bias": bias_pack, "h0t": h0t_pack, "c0b": c0,
            "wcls": wcls_pack,
        })

    res = run_bass_kernel_spmd(nc, in_maps, list(range(NCORES)), trace=TRACE)
    if TRACE and res.exec_time_ns is not None:
        print(f"HW exec time: {res.exec_time_ns} ns")

    out_full = np.empty((B, T, V), np.float32)
    for c in range(NCORES):
        lt = res.results[c]["outp"].reshape(VS, T, B)   # [v, t, b]
        out_full[:, :, c * VREAL:(c + 1) * VREAL] = lt[:VREAL].transpose(2, 1, 0)

    hT_buf = res.results[0]["hT"]                       # [L, 128, 128]
    h_out = np.stack([
        hT_buf[l].reshape(128, 4, 32).transpose(2, 1, 0).reshape(32, 512)
        for l in range(L)])
    c_out = res.results[0]["cT"]                        # [L, 32, 512]

    return (out_full, h_out, c_out)


# revision 7
# speedup vs baseline: 1.3738x; 1.0426x over previous
"""Trainium2 Bass kernel for nn_Decoder_70549132804202.

4-layer LSTM decoder (B=32, T=64, H=E=512) + 32000-way classifier with
log_softmax over the sequence axis, SPMD across 8 NeuronCores.

Phase-1 distribution: the LSTM scan is replicated on all 8 cores (it is
latency-bound and cheap to replicate); the classifier weight / vocab dim
is sharded 8 ways (log_softmax is over T, so it is fully local per core);
each core writes its [4096, 64*32] v-major slice of the output, assembled
on the host.

Layouts (per core):
  - free index n = t*32 + b   (t-major columns everywhere)
  - inpT / outsT: one [128, 8192] SBUF tile, cols = k*2048 + n  (h-dim
    tile k on partitions)
  - weights WihT/WhhT: [128, 8192] cols = k*2048 + gate_col, gate cols
    permuted to [i | f | o | g] so one Sigmoid op covers i,f,o.
  - gates computed B-major [32, 2048] in PSUM; h' transposed back to
    H-major via PE transposes each step.
"""

import os
import numpy as np
import ml_dtypes

BF16 = ml_dtypes.bfloat16

V, E, H, L, B, T = 32000, 512, 512, 4, 32, 64
NT = B * T                    # 2048
VPAD = 32768
NCORES = 8
VS = VPAD // NCORES           # 4096 rows/core (padded)
VREAL = V // NCORES           # 4000 real rows/core

TRACE = bool(int(os.environ.get("BASS_KERNEL_TRACE", "0")))

_cache = {}


def _split_excess_waits(nc, limit=1):
    """This container's walrus (2026-05-04) rejects instructions with more
    than one sync-wait command.  Hoist excess waits onto InstNoOp's
    inserted just before, on the same engine (engine streams execute in
    block order, so the waits still complete before the instruction)."""
    import concourse.mybir as mybir

    n_split = 0
    for bb in nc.main_func.blocks:
        il = bb.instructions
        i = 0
        while i < len(il):
            ins = il[i]
            si = getattr(ins, "sync_info", None)
            if si is not None and si.on_wait is not None and len(si.on_wait) > limit:
                waits = list(si.on_wait)
                keep, extra = waits[-limit:], waits[:-limit]
                pos = i
                for c0 in range(0, len(extra), limit):
                    chunk = extra[c0 : c0 + limit]
                    nop = mybir.InstNoOp(name=f"{ins.name}-ws{c0}", ins=[], outs=[])
                    nop.engine = ins.engine
                    nop.sync_info = mybir.SyncInfo(on_update=[], on_wait=chunk)
                    il.insert(pos, nop)
                    pos += 1
                    i += 1
                si.on_wait = keep
                n_split += 1
            i += 1
    return n_split


def _build():
    import concourse.bass as bass
    import concourse.mybir as mybir
    import concourse.tile as tile
    from concourse.masks import make_identity

    f32 = mybir.dt.float32
    bf16 = mybir.dt.bfloat16
    i32 = mybir.dt.int32
    AF = mybir.ActivationFunctionType
    OP = mybir.AluOpType
    AX = mybir.AxisListType

    nc = bass.Bass("TRN2", target_bir_lowering=False, debug=False,
                   num_devices=NCORES)

    emb_ext = nc.declare_dram_parameter("emb", [V, E], f32, isOutput=False)
    dec_ext = nc.declare_dram_parameter("dec", [16, 128, 1], i32, isOutput=False)
    wih_ext = nc.declare_dram_parameter("wih", [L, 128, 8192], bf16, isOutput=False)
    whh_ext = nc.declare_dram_parameter("whh", [L, 128, 8192], bf16, isOutput=False)
    bias_ext = nc.declare_dram_parameter("bias", [L, 1, 2048], bf16, isOutput=False)
    h0_ext = nc.declare_dram_parameter("h0t", [L, 128, 128], bf16, isOutput=False)
    c0_ext = nc.declare_dram_parameter("c0b", [L, 32, 512], f32, isOutput=False)
    wcls_ext = nc.declare_dram_parameter("wcls", [32, 128, 512], bf16, isOutput=False)

    out_ext = nc.declare_dram_parameter("outp", [32, 128, 2048], f32, isOutput=True)
    hT_ext = nc.declare_dram_parameter("hT", [L, 128, 128], f32, isOutput=True)
    cT_ext = nc.declare_dram_parameter("cT", [L, 32, 512], f32, isOutput=True)

    with tile.TileContext(nc) as tc:
        with (
            tc.tile_pool(name="inpT", bufs=2) as p_inpT,
            tc.tile_pool(name="wts", bufs=2) as p_w,
            tc.tile_pool(name="xg", bufs=4) as p_xg,
            tc.tile_pool(name="big", bufs=3) as p_big,
            tc.tile_pool(name="chain", bufs=2) as p_chain,
            tc.tile_pool(name="state", bufs=2) as p_state,
            tc.tile_pool(name="wc", bufs=2) as p_wc,
            tc.tile_pool(name="misc", bufs=1) as p_misc,
            tc.tile_pool(name="ps", bufs=1, space="PSUM") as p_ps,
            tc.tile_pool(name="pstr", bufs=2, space="PSUM") as p_pstr,
            tc.tile_pool(name="psb", bufs=1, space="PSUM") as p_psb,
            tc.tile_pool(name="dram", bufs=2, space="DRAM") as p_dram,
        ):
            ident = p_misc.tile([128, 128], f32, tag="ident")
            make_identity(nc, ident[:])
            ones = p_misc.tile([1, 128], bf16, tag="ones")
            nc.gpsimd.memset(ones[:], 1.0)
            ident_bf = p_misc.tile([32, 32], bf16, tag="identbf")
            nc.vector.tensor_copy(ident_bf[:], ident[0:32, 0:32])
            bias_sb = p_misc.tile([1, 2048], bf16, tag="bias")

            # ---- Stage A: embedding gather + transpose -> inpT (layer-0 input) ----
            inpT = p_inpT.tile([128, 8192], bf16, tag="inpT")
            for g in range(16):
                idx = p_wc.tile([128, 1], i32, tag="idx")
                nc.sync.dma_start(idx[:], dec_ext[g])
                xr = p_big.tile([128, 512], f32, tag="big")
                nc.gpsimd.indirect_dma_start(
                    out=xr[:], out_offset=None,
                    in_=emb_ext[:],
                    in_offset=bass.IndirectOffsetOnAxis(ap=idx[:, :1], axis=0),
                )
                for k in range(4):
                    pst = p_pstr.tile([128, 128], f32, tag="pstr")
                    nc.tensor.transpose(out=pst[:], in_=xr[:, k * 128:(k + 1) * 128],
                                        identity=ident[:])
                    nc.vector.tensor_copy(
                        out=inpT[:, k * 2048 + g * 128: k * 2048 + g * 128 + 128],
                        in_=pst[:])

            # ---- layers ----
            # B1 (batched x-side gates) for layer l+1 is interleaved into
            # layer l's scan so PE fills the recurrence gaps.
            def emit_b1_half(mc, half, src_inpT, w_sb, xgd):
                    psb = p_psb.tile([128, 1024], f32, tag="psb")
                    for n2 in range(2):
                        n = half * 2 + n2
                        nsl = slice(n2 * 512, (n2 + 1) * 512)
                        for k in range(4):
                            nc.tensor.matmul(
                                psb[:, nsl],
                                lhsT=src_inpT[:, k * 2048 + mc * 128: k * 2048 + mc * 128 + 128],
                                rhs=w_sb[:, k * 2048 + n * 512: k * 2048 + (n + 1) * 512],
                                start=(k == 0), stop=False)
                        nc.tensor.matmul(psb[:, nsl], lhsT=ones[0:1, 0:128],
                                         rhs=bias_sb[0:1, n * 512:(n + 1) * 512],
                                         start=False, stop=True)
                    st = p_big.tile([128, 1024], bf16, tag="bigb")
                    nc.scalar.copy(st[:], psb[:])
                    nc.sync.dma_start(xgd[mc, :, half * 1024:(half + 1) * 1024], st[:])

            def emit_b1_chunk(mc, src_inpT, w_sb, xgd):
                for half in range(2):
                    emit_b1_half(mc, half, src_inpT, w_sb, xgd)

            # layer-0 x-gates up front
            w_ih_sb = p_w.tile([128, 8192], bf16, tag="w")
            nc.sync.dma_start(w_ih_sb[:], wih_ext[0])
            nc.sync.dma_start(bias_sb[:], bias_ext[0])
            xgd = p_dram.tile([16, 128, 2048], bf16, tag="xg")
            for mc in range(16):
                emit_b1_chunk(mc, inpT, w_ih_sb, xgd)

            for l in range(L):
                # B2: recurrent scan for layer l (+ interleaved B1 for l+1)
                w_sb = p_w.tile([128, 8192], bf16, tag="w")
                nc.sync.dma_start(w_sb[:], whh_ext[l])
                hT = p_state.tile([128, 128], bf16, tag="hT")
                nc.sync.dma_start(hT[:], h0_ext[l])
                cB = p_state.tile([32, 512], f32, tag="cB")
                nc.sync.dma_start(cB[:], c0_ext[l])
                inpT_next = p_inpT.tile([128, 8192], bf16, tag="inpT")
                if l + 1 < L:
                    w_ih_sb = p_w.tile([128, 8192], bf16, tag="w")
                    nc.sync.dma_start(w_ih_sb[:], wih_ext[l + 1])
                    nc.sync.dma_start(bias_sb[:], bias_ext[l + 1])
                    xgd_next = p_dram.tile([16, 128, 2048], bf16, tag="xg")
                b1_jobs = [(mc, h) for mc in range(16) for h in range(2)]
                b1_ji = 0

                for t in range(T):
                    xg = p_xg.tile([32, 2048], bf16, tag="xg")
                    nc.sync.dma_start(
                        xg[:], xgd[t // 4, (t % 4) * 32:(t % 4) * 32 + 32, :])
                    ps = p_ps.tile([32, 2048], f32, tag="ps")
                    for n in range(4):
                        ns = slice(n * 512, (n + 1) * 512)
                        nc.tensor.matmul(ps[:, ns], lhsT=ident_bf[:],
                                         rhs=xg[:, ns], start=True, stop=False)
                        for k in range(4):
                            nc.tensor.matmul(
                                ps[:, ns], lhsT=hT[:, k * 32:(k + 1) * 32],
                                rhs=w_sb[:, k * 2048 + n * 512: k * 2048 + (n + 1) * 512],
                                start=False, stop=(k == 3))
                    # gate math (gate cols permuted to [i | f | o | g])
                    sigf = p_chain.tile([32, 512], f32, tag="sigf")
                    nc.scalar.activation(sigf[:], ps[:, 512:1024], AF.Sigmoid)
                    tg = p_chain.tile([32, 512], f32, tag="tg")
                    nc.scalar.activation(tg[:], ps[:, 1536:2048], AF.Tanh)
                    sig = p_chain.tile([32, 1024], f32, tag="sig")
                    nc.scalar.activation(sig[:, 0:512], ps[:, 0:512], AF.Sigmoid)
                    nc.scalar.activation(sig[:, 512:1024], ps[:, 1024:1536], AF.Sigmoid)
                    m1 = p_chain.tile([32, 512], f32, tag="m1")
                    nc.vector.tensor_tensor(out=m1[:], in0=sigf[:],
                                            in1=cB[:], op=OP.mult)
                    m2 = p_chain.tile([32, 512], f32, tag="m2")
                    nc.vector.tensor_tensor(out=m2[:], in0=sig[:, 0:512],
                                            in1=tg[:], op=OP.mult)
                    cB = p_state.tile([32, 512], f32, tag="cB")
                    nc.vector.tensor_tensor(out=cB[:], in0=m1[:], in1=m2[:], op=OP.add)
                    tcl = p_chain.tile([32, 512], f32, tag="tc")
                    nc.scalar.activation(tcl[:], cB[:], AF.Tanh)
                    hp = p_chain.tile([32, 512], f32, tag="hp")
                    nc.vector.tensor_tensor(out=hp[:], in0=sig[:, 512:1024],
                                            in1=tcl[:], op=OP.mult)
                    # h' -> H-major (new hT) and residual add -> inpT_next cols
                    pst = p_pstr.tile([128, 128], f32, tag="pstr")
                    for k in range(4):
                        nc.tensor.transpose(out=pst[:, k * 32:(k + 1) * 32],
                                            in_=hp[:, k * 128:(k + 1) * 128],
                                            identity=ident[0:32, 0:32])
                    hT = p_state.tile([128, 128], bf16, tag="hT")
                    nc.vector.tensor_copy(out=hT[:], in_=pst[:])
                    pst_v = pst[:].rearrange("p (k n) -> p k n", k=4)
                    in_v = inpT[:].rearrange("p (k n) -> p k n", k=4)[:, :, t * 32:(t + 1) * 32]
                    out_v = inpT_next[:].rearrange("p (k n) -> p k n", k=4)[:, :, t * 32:(t + 1) * 32]
                    with nc.allow_low_precision("bf16 activation stream"):
                        nc.vector.tensor_tensor(out=out_v, in0=pst_v, in1=in_v, op=OP.add)
                    if t == T - 1:
                        hT_f = p_state.tile([128, 128], f32, tag="hTf")
                        nc.vector.tensor_copy(out=hT_f[:], in_=pst[:])
                    if (l + 1 < L and b1_ji < 32
                            and b1_jobs[b1_ji][0] * 4 + 3 <= t):
                        emit_b1_half(*b1_jobs[b1_ji], inpT_next, w_ih_sb,
                                     xgd_next)
                        b1_ji += 1

                while l + 1 < L and b1_ji < 32:
                    emit_b1_half(*b1_jobs[b1_ji], inpT_next, w_ih_sb, xgd_next)
                    b1_ji += 1

                nc.sync.dma_start(hT_ext[l], hT_f[:])
                nc.sync.dma_start(cT_ext[l], cB[:])
                inpT = inpT_next
                if l + 1 < L:
                    xgd = xgd_next

            # ---- classifier + log_softmax over T (V-sharded; b_cls cancels) ----
            for vt in range(32):
                wc = p_wc.tile([128, 512], bf16, tag="wc")
                nc.sync.dma_start(wc[:], wcls_ext[vt])
                ps = p_ps.tile([128, 2048], f32, tag="ps")
                for n in range(4):
                    ns = slice(n * 512, (n + 1) * 512)
                    for k in range(4):
                        nc.tensor.matmul(
                            ps[:, ns], lhsT=wc[:, k * 128:(k + 1) * 128],
                            rhs=inpT[:, k * 2048 + n * 512: k * 2048 + (n + 1) * 512],
                            start=(k == 0), stop=(k == 3))
                lg = p_big.tile([128, 2048], f32, tag="big")
                nc.scalar.copy(lg[:], ps[:])
                ex = p_big.tile([128, 2048], f32, tag="big")
                nc.scalar.activation(ex[:], lg[:], AF.Exp)
                ls = p_wc.tile([128, 32], f32, tag="ls")
                nc.vector.tensor_reduce(out=ls[:],
                                        in_=ex[:].rearrange("p (t b) -> p b t", b=32),
                                        axis=AX.X, op=OP.add)
                lse = p_wc.tile([128, 32], f32, tag="lse")
                nc.scalar.activation(lse[:], ls[:], AF.Ln)
                nc.vector.tensor_tensor(
                    out=ex[:].rearrange("p (t b) -> p b t", b=32),
                    in0=lg[:].rearrange("p (t b) -> p b t", b=32),
                    in1=lse[:].to_broadcast([128, 32, 64]),
                    op=OP.subtract)
                nc.sync.dma_start(out_ext[vt], ex[:])

    _split_excess_waits(nc, 1)
    return nc


def _get_nc():
    if "nc" not in _cache:
        _cache["nc"] = _build()
    return _cache["nc"]


def _pack_kT(WT):
    """[512, C] (h-dim major) -> [128, 4*C] with cols = k*C + c."""
    C = WT.shape[1]
    return np.ascontiguousarray(
        WT.reshape(4, 128, C).transpose(1, 0, 2).reshape(128, 4 * C))


def kernel(context=None, dec_input=None, h0=None, c0=None, emb=None,
           W_ih=None, W_hh=None, b_ih=None, b_hh=None, W_cls=None,
           b_cls=None):
    from concourse.bass_utils import run_bass_kernel_spmd

    nc = _get_nc()

    emb = np.ascontiguousarray(np.asarray(emb, np.float32))
    dec = np.asarray(dec_input)
    h0 = np.asarray(h0, np.float32)
    c0 = np.ascontiguousarray(np.asarray(c0, np.float32))
    W_ih = np.asarray(W_ih, np.float32)
    W_hh = np.asarray(W_hh, np.float32)
    b_ih = np.asarray(b_ih, np.float32)
    b_hh = np.asarray(b_hh, np.float32)
    W_cls = np.asarray(W_cls, np.float32)

    # gate permutation: torch order [i f g o] -> kernel order [i f o g]
    perm = np.concatenate([np.arange(0, 512), np.arange(512, 1024),
                           np.arange(1536, 2048), np.arange(1024, 1536)])

    wih_pack = np.stack([_pack_kT(W_ih[l][perm].T) for l in range(L)]).astype(BF16)
    whh_pack = np.stack([_pack_kT(W_hh[l][perm].T) for l in range(L)]).astype(BF16)
    bias_pack = np.ascontiguousarray(
        (b_ih + b_hh)[:, perm].reshape(L, 1, 2048).astype(BF16))
    h0t_pack = np.stack([_pack_kT(h0[l].T) for l in range(L)]).astype(BF16)

    # n = t*32 + b  ->  dec.T flattened
    dec_pack = np.ascontiguousarray(
        dec.T.reshape(16, 128, 1).astype(np.int32))

    in_maps = []
    for c in range(NCORES):
        shard = np.zeros((VS, H), np.float32)       # [4096, 512]
        shard[:VREAL] = W_cls[c * VREAL:(c + 1) * VREAL]
        wcls_pack = np.stack([_pack_kT(shard[vt * 128:(vt + 1) * 128].T)
                              for vt in range(32)]).astype(BF16)
        in_maps.append({
            "emb": emb, "dec": dec_pack, "wih": wih_pack, "whh": whh_pack,
            "bias": bias_pack, "h0t": h0t_pack, "c0b": c0,
            "wcls": wcls_pack,
        })

    res = run_bass_kernel_spmd(nc, in_maps, list(range(NCORES)), trace=TRACE)
    if TRACE and res.exec_time_ns is not None:
        print(f"HW exec time: {res.exec_time_ns} ns")

    out_full = np.empty((B, T, V), np.float32)
    for c in range(NCORES):
        lt = res.results[c]["outp"].reshape(VS, T, B)   # [v, t, b]
        out_full[:, :, c * VREAL:(c + 1) * VREAL] = lt[:VREAL].transpose(2, 1, 0)

    hT_buf = res.results[0]["hT"]                       # [L, 128, 128]
    h_out = np.stack([
        hT_buf[l].reshape(128, 4, 32).transpose(2, 1, 0).reshape(32, 512)
        for l in range(L)])
    c_out = res.results[0]["cT"]                        # [L, 32, 512]

    return (out_full, h_out, c_out)
